# revision 36
# baseline (speedup 1.0000x reference)
"""Multi-head attention Trainium2 kernel (8 NeuronCores, SPMD).

Problem: B=2, S=2048, D=1024, H=16 heads, DK=DV=64.
Sharding: batch (2) x head-groups (4 heads per core) = 8 cores.
Each core computes, for its batch b and its 4 heads:
    Q/K/V projections, attention, and the partial output projection
    out_partial = concat_heads(ctx) @ Wo[head rows].
Host gathers by summing the 4 partials per batch and adding bo.

v4 design (405us baseline -> 298us v2 -> this). All matmuls bf16/fp32-psum:
fp8 anywhere on the q/k or value paths exceeds the 2e-2 max-rel-err budget
(measured 1e-2 for fp8 q/k alone, 3.5e-2 for fp8 ctx/Wo), and fp8 DoubleRow
gave no per-instruction speedup on this hardware anyway (~450ns vs ~340ns
per 512-row matmul).

Keys to the speedup over v2:
  - The PE runs at full clock only when continuously fed (the projection
    phase hits 2.4GHz; the v2 attention loop sagged to ~1.4GHz because the
    per-group exp dependency starves it ~200ns per group and the clock
    drops). The output projection of chunk c-1 is therefore INTERLEAVED
    into chunk c's attention stream as always-ready filler work.
  - exp is split across engines: groups 0,2,4,6 use exact Act-engine Exp;
    groups 1,3,5,7 use a DVE fast-exp (int16(s*23.083+16249.64) bit
    pattern read as bf16 ~= exp(s/8), 1.7% mean err; washes out in the
    2048-way softmax averages - measured end-to-end 7e-3).
  - 1/sqrt(DK) folds into the exp scale (and the fast-exp multiplier).
  - Softmax denominator comes free from 64 ones-columns appended to V
    (PSUM rows 64..127 of the ctx accumulator); its reciprocal is
    exp(-ln(den)) on Act - ln/exp share one activation table (no reload),
    vs 3.4us per tile for DVE RECIPROCAL.
  - x arrives host-pre-transposed in bf16; V is computed directly in
    [t,(h,v)] layout (stationary = xT tile) - no PE transposes at all.
  - Q/K-projection PSUM evacuation alternates Act (Identity+bias) / DVE
    (tensor_scalar_add); outproj evacuation alternates Act/DVE copies;
    bo is added by the host during the gather.
"""
import sys

if "/opt/trn_rl_repo" not in sys.path:
    sys.path.insert(0, "/opt/trn_rl_repo")

import ml_dtypes
import numpy as np

import bass_rust
import concourse.bass as bass
import concourse.mybir as mybir
import concourse.tile as tile
from concourse.bass_utils import run_bass_kernel_spmd
from concourse.vector_clock import ScopedClock

F32 = mybir.dt.float32
BF16 = mybir.dt.bfloat16
I16 = mybir.dt.int16
AF = mybir.ActivationFunctionType
Alu = mybir.AluOpType
BF16NP = ml_dtypes.bfloat16

B, S, D = 2, 2048, 1024
H, DK, DV = 16, 64, 64
HL = 4          # heads per core
NPAIR = 2
ST = S // 128   # 16
DT = D // 128   # 8
SC = 512        # attention s-chunk
NSC = S // SC   # 4
N_CORES = 8

# int16(score*FEXP_A + FEXP_B) bit pattern read as bf16 ~= exp(score/8)
FEXP_A = 0.125 * 128.0 * 1.4426950408889634
FEXP_B = 16249.64
ACT_GROUPS = (0, 2, 4, 6)   # exact Act exp; remaining groups use DVE fastexp


class _TileContextSplitDrain(tile.TileContext):
    """Walrus in this container rejects ANY instruction carrying >1 sem wait
    ("Too many sync wait commands"). Post-lowering, sweep every basic block
    and move surplus waits onto injected EventSemaphore carrier instructions
    placed immediately before the over-subscribed instruction (same engine,
    same program point - semantics unchanged)."""

    _MAXW = 1

    def _split_excess_waits(self):
        nc = self.nc
        for fn in nc.m.functions:
            for bb in fn.blocks:
                insts = bb.instructions
                new_list = []
                changed = False
                for ins in insts:
                    si = ins.sync_info
                    waits = list(si.on_wait) if si is not None and si.on_wait else []
                    if len(waits) > self._MAXW:
                        changed = True
                        extra, keep = waits[:-self._MAXW], waits[-self._MAXW:]
                        for k in range(0, len(extra), self._MAXW):
                            chunk = extra[k:k + self._MAXW]
                            ev = mybir.InstEventSemaphore(
                                name=f"wsplit_{nc.next_id()}", ins=[], outs=[]
                            )
                            ev.engine = ins.engine
                            ev.sync_info = bass_rust.SyncInfo(
                                on_wait=chunk, on_update=[]
                            )
                            nc.register_instruction(ev, overwrite=True)
                            new_list.append(ev)
                        ins.sync_info = bass_rust.SyncInfo(
                            on_wait=keep,
                            on_update=list(si.on_update) if si.on_update else [],
                        )
                    new_list.append(ins)
                if changed:
                    insts[:] = new_list

    def _drain_and_barrier(self, tick_clock, wait_clock):
        self._split_excess_waits()
        ticks = list(tick_clock.global_clock)
        for p, t in enumerate(ticks):
            if t <= 0:
                continue
            v = bass_rust.VectorClock()
            v.require_at_least(p, t)
            d = self.nc.sync.drain()
            wait_clock.add_sem_waits(d.ins, ScopedClock({None: v}))
        self.nc.all_engine_barrier()
        popped = self.nc._tile_sem_poison_stack.pop()
        assert popped is self._sem_poison
        self.nc.clear_and_free_semaphores(list(self.sems.allocated().values()))
        self.nc.all_engine_barrier()


def build_nc() -> bass.Bass:
    nc = bass.Bass()

    # all inputs partition-major: row p holds everything partition p needs,
    # contiguously, so each dma_start is 128 large descriptors
    xT_d = nc.dram_tensor("xT", [128, NSC * DT * 512], BF16, kind="ExternalInput")
    wqk_d = nc.dram_tensor("wqk", [128, 4 * DT * 128], BF16, kind="ExternalInput")
    wv_d = nc.dram_tensor("wv", [128, DT * HL * DV], BF16, kind="ExternalInput")
    wo_d = nc.dram_tensor("wo", [128, NPAIR * D], BF16, kind="ExternalInput")
    bqk_d = nc.dram_tensor("bqk", [128, 4], F32, kind="ExternalInput")
    bv_d = nc.dram_tensor("bv", [1, HL * DV], F32, kind="ExternalInput")
    out_d = nc.dram_tensor("out", [S, D], F32, kind="ExternalOutput")

    with _TileContextSplitDrain(nc) as tc:
        with (
            tc.tile_pool(name="const", bufs=1) as constp,
            tc.tile_pool(name="persist", bufs=1) as pers,
        ):
            wqk_sb = constp.tile([128, DT, 4 * 128], BF16, tag="wqk")
            wv_sb = constp.tile([128, DT, HL * DV], BF16, tag="wv")
            wo_sb = constp.tile([128, NPAIR, D], BF16, tag="wo")
            bqk_sb = constp.tile([128, 4], F32, tag="bqk")
            bv_rep = constp.tile([128, HL * DV], F32, tag="bv_rep")
            xT_sb = pers.tile([128, DT, S], BF16, tag="xT")
            QT = pers.tile([128, NPAIR, S], BF16, tag="QT")
            KT = pers.tile([128, NPAIR, S], BF16, tag="KT")
            ctxT = pers.tile([128, NPAIR, S], BF16, tag="ctxT")
            V_aug = pers.tile([128, ST, HL, 128], BF16, tag="V_aug")

            def dma_xT(sc):
                nc.sync.dma_start(
                    out=xT_sb[:, :, sc * 512:(sc + 1) * 512],
                    in_=xT_d.rearrange("p (c d s) -> p c d s", c=NSC, d=DT)[:, sc, :, :],
                )

            wqk_r = wqk_d.rearrange("p (cb d c) -> p cb d c", cb=4, d=DT)
            dma_xT(0)
            nc.sync.dma_start(out=wqk_sb[:, :, 0:128], in_=wqk_r[:, 0, :, :])
            nc.sync.dma_start(out=wqk_sb[:, :, 128:256], in_=wqk_r[:, 1, :, :])
            dma_xT(1)
            nc.sync.dma_start(out=wqk_sb[:, :, 256:384], in_=wqk_r[:, 2, :, :])
            nc.sync.dma_start(out=wqk_sb[:, :, 384:512], in_=wqk_r[:, 3, :, :])
            nc.sync.dma_start(out=wv_sb, in_=wv_d[:, :])
            nc.sync.dma_start(out=bqk_sb, in_=bqk_d[:, :])
            nc.sync.dma_start(out=bv_rep, in_=bv_d[0:1, :].to_broadcast((128, HL * DV)))
            nc.vector.memset(V_aug[:, :, :, 64:128], 1.0)

            # ---------------- Phase 1: QKV projections ----------------------
            with (
                tc.tile_pool(name="qkps", bufs=5, space="PSUM") as qkp,
                tc.tile_pool(name="vps", bufs=3, space="PSUM") as vp,
            ):
                for sc in range(4):
                    if sc + 2 < 4:
                        dma_xT(sc + 2)
                    if sc == 1:
                        nc.sync.dma_start(out=wo_sb, in_=wo_d[:, :])
                    for proj, dest in ((0, QT), (1, KT)):
                        for pair in range(NPAIR):
                            col = (2 * proj + pair) * 128
                            ps = qkp.tile([128, 512], F32, tag="qkps")
                            for d in range(DT):
                                nc.tensor.matmul(
                                    ps[:, :],
                                    wqk_sb[:, d, col:col + 128],
                                    xT_sb[:, d, sc * 512:(sc + 1) * 512],
                                    start=(d == 0),
                                    stop=(d == DT - 1),
                                )
                            dst = dest[:, pair, sc * 512:(sc + 1) * 512]
                            bias = bqk_sb[:, 2 * proj + pair:2 * proj + pair + 1]
                            if pair == 0:
                                nc.scalar.activation(
                                    out=dst, in_=ps[:, :], func=AF.Identity,
                                    bias=bias, scale=1.0,
                                )
                            else:
                                nc.vector.tensor_scalar_add(dst, ps[:, :], bias)
                    for tt in range(4):
                        j = 4 * sc + tt
                        vps = vp.tile([128, HL * DV], F32, tag="vps")
                        for d in range(DT):
                            nc.tensor.matmul(
                                vps[:, :],
                                xT_sb[:, d, j * 128:(j + 1) * 128],
                                wv_sb[:, d, :],
                                start=(d == 0),
                                stop=(d == DT - 1),
                            )
                        nc.vector.tensor_add(
                            V_aug[:, j, :, 0:64],
                            vps.rearrange("p (h v) -> p h v", h=HL),
                            bv_rep.rearrange("p (h v) -> p h v", h=HL),
                        )

            # ---------------- Phase 2: attention with interleaved outproj ---
            with (
                tc.tile_pool(name="sps", bufs=2, space="PSUM") as spp,
                tc.tile_pool(name="cpp", bufs=2, space="PSUM") as cpp,
                tc.tile_pool(name="opp", bufs=2, space="PSUM") as opp,
                tc.tile_pool(name="ptp", bufs=8) as ptp,
                tc.tile_pool(name="recl", bufs=4) as recl,
                tc.tile_pool(name="recp", bufs=4) as recp,
                tc.tile_pool(name="otp", bufs=6) as otp,
            ):
                pending = []   # outproj steps of the previous chunk
                ot_ref = [None]

                def outproj_step(st, dc):
                    if dc == 0:
                        ot_ref[0] = otp.tile([128, D], F32, name="ot", tag="ot")
                    ot = ot_ref[0]
                    ops_t = opp.tile([128, 512], F32, name="ops", tag="ops")
                    for pair in range(NPAIR):
                        nc.tensor.matmul(
                            ops_t[:, :],
                            ctxT[:, pair, st * 128:(st + 1) * 128],
                            wo_sb[:, pair, dc * 512:(dc + 1) * 512],
                            start=(pair == 0),
                            stop=(pair == NPAIR - 1),
                        )
                    if dc == 0:
                        nc.scalar.copy(ot[:, 0:512], ops_t[:, :])
                    else:
                        nc.vector.tensor_copy(ot[:, 512:1024], ops_t[:, :])
                        nc.sync.dma_start(
                            out=out_d[st * 128:(st + 1) * 128, :], in_=ot
                        )

                for c in range(NSC):
                    for pair in range(NPAIR):
                        for e in range(2):
                            h = 2 * pair + e
                            base = 64 * e
                            cp = cpp.tile([128, SC], F32, name="cp", tag="cp")
                            sps = {}

                            def emit_scores(g, _pair=pair, _base=base, _c=c):
                                sp = spp.tile([128, 2, SC], F32, name="sp", tag="sp")
                                for k in range(2):
                                    j = 2 * g + k
                                    nc.tensor.matmul(
                                        sp[:, k, :],
                                        KT[_base:_base + 64, _pair, j * 128:(j + 1) * 128],
                                        QT[_base:_base + 64, _pair, _c * SC:(_c + 1) * SC],
                                        start=True,
                                        stop=True,
                                    )
                                sps[g] = sp

                            emit_scores(0)
                            emit_scores(1)
                            for g in range(8):
                                if g + 2 < 8:
                                    emit_scores(g + 2)
                                sp = sps.pop(g)
                                pt = ptp.tile([128, 2, SC], BF16, tag="pt")
                                if g in ACT_GROUPS:
                                    nc.scalar.activation(
                                        out=pt[:, :, :], in_=sp[:, :, :],
                                        func=AF.Exp, scale=0.125,
                                    )
                                else:
                                    nc.vector.tensor_scalar(
                                        pt.bitcast(I16), sp[:, :, :],
                                        FEXP_A, FEXP_B, Alu.mult, Alu.add,
                                    )
                                for k in range(2):
                                    j = 2 * g + k
                                    nc.tensor.matmul(
                                        cp[:, :],
                                        V_aug[:, j, h, :],
                                        pt[:, k, :],
                                        start=(g == 0 and k == 0),
                                        stop=(g == 7 and k == 1),
                                    )
                                if g % 2 == 1 and pending:
                                    pending.pop(0)()
                            # 1/den = exp(-ln(den)); den sits on psum rows
                            # 64..127 via the ones-columns of V_aug
                            rl = recl.tile([64, SC], F32, tag="rl")
                            nc.scalar.activation(out=rl, in_=cp[64:128, :], func=AF.Ln)
                            rec = recp.tile([64, SC], F32, tag="rec")
                            nc.scalar.activation(out=rec, in_=rl, func=AF.Exp, scale=-1.0)
                            nc.vector.tensor_mul(
                                ctxT[base:base + 64, pair, c * SC:(c + 1) * SC],
                                cp[0:64, :],
                                rec,
                            )
                    for st in range(4 * c, 4 * c + 4):
                        for dc in range(2):
                            pending.append(
                                lambda _st=st, _dc=dc: outproj_step(_st, _dc)
                            )
                while pending:
                    pending.pop(0)()

    return nc


_NC_CACHE = None


def get_nc() -> bass.Bass:
    global _NC_CACHE
    if _NC_CACHE is None:
        _NC_CACHE = build_nc()
    return _NC_CACHE


def prep_in_maps(hidden_state, Wq, bq, Wk, bk, Wv, bv, Wo, bo):
    hidden_state = np.asarray(hidden_state, np.float32)
    Wq, bq = np.asarray(Wq, np.float32), np.asarray(bq, np.float32)
    Wk, bk = np.asarray(Wk, np.float32), np.asarray(bk, np.float32)
    Wv, bv = np.asarray(Wv, np.float32), np.asarray(bv, np.float32)
    Wo, bo = np.asarray(Wo, np.float32), np.asarray(bo, np.float32)

    in_maps = []
    for core in range(N_CORES):
        b, g = core // 4, core % 4
        hs = slice(HL * g, HL * (g + 1))
        # [D, C] -> [128, DT, C]: row p holds d-tiles d*128+p contiguously
        def pmaj(a):
            return np.ascontiguousarray(
                a.reshape(DT, 128, -1).transpose(1, 0, 2)
            ).reshape(128, -1)

        xT = hidden_state[b].T.astype(BF16NP)
        # [128, NSC, DT, 512]
        xT = np.ascontiguousarray(
            xT.reshape(DT, 128, NSC, 512).transpose(1, 2, 0, 3)
        ).reshape(128, -1)
        cols = []
        for wmat in (Wq[hs], Wk[hs]):
            for pair in range(NPAIR):
                cols.append(
                    wmat[2 * pair:2 * pair + 2].transpose(1, 0, 2).reshape(D, 128)
                )
        # [D, 4*128] -> [128, colblk, DT, 128] partition-major per block
        wqk = np.ascontiguousarray(
            np.concatenate(cols, axis=1).astype(BF16NP)
            .reshape(DT, 128, 4, 128).transpose(1, 2, 0, 3)
        ).reshape(128, -1)
        wv_g = pmaj(Wv[hs].transpose(1, 0, 2).reshape(D, HL * DV).astype(BF16NP))
        wo_g = np.ascontiguousarray(
            Wo[HL * DV * g: HL * DV * (g + 1)].astype(BF16NP)
            .reshape(NPAIR, 128, D).transpose(1, 0, 2)
        ).reshape(128, -1)
        bqk_cols = []
        for bvec in (bq[hs], bk[hs]):
            for pair in range(NPAIR):
                bqk_cols.append(bvec[2 * pair:2 * pair + 2].reshape(128))
        bqk = np.stack(bqk_cols, axis=1).astype(np.float32)
        in_maps.append({
            "xT": xT,
            "wqk": np.ascontiguousarray(wqk),
            "wv": np.ascontiguousarray(wv_g),
            "wo": wo_g,
            "bqk": np.ascontiguousarray(bqk),
            "bv": np.ascontiguousarray(bv[hs].reshape(1, HL * DV)),
        })
    return in_maps


_BO = None


def gather(results):
    """Sum the 4 row-parallel partials per batch, then add bo."""
    out = np.empty((B, S, D), np.float32)
    for b in range(B):
        acc = results[4 * b]["out"].astype(np.float32)
        for g in range(1, 4):
            acc = acc + results[4 * b + g]["out"]
        out[b] = acc + _BO[None, :]
    return out


def kernel(**inputs) -> np.ndarray:
    global _BO
    _BO = np.asarray(inputs["bo"], np.float32)
    nc = get_nc()
    in_maps = prep_in_maps(**inputs)
    res = run_bass_kernel_spmd(nc, in_maps, core_ids=list(range(N_CORES)))
    return gather(res.results)


# revision 37
# speedup vs baseline: 1.0201x; 1.0201x over previous
"""Multi-head attention Trainium2 kernel (8 NeuronCores, SPMD).

Problem: B=2, S=2048, D=1024, H=16 heads, DK=DV=64.
Sharding: batch (2) x head-groups (4 heads per core) = 8 cores.
Each core computes, for its batch b and its 4 heads:
    Q/K/V projections, attention, and the partial output projection
    out_partial = concat_heads(ctx) @ Wo[head rows].
Host gathers by summing the 4 partials per batch and adding bo.

v4 design (405us baseline -> 298us v2 -> this). All matmuls bf16/fp32-psum:
fp8 anywhere on the q/k or value paths exceeds the 2e-2 max-rel-err budget
(measured 1e-2 for fp8 q/k alone, 3.5e-2 for fp8 ctx/Wo), and fp8 DoubleRow
gave no per-instruction speedup on this hardware anyway (~450ns vs ~340ns
per 512-row matmul).

Keys to the speedup over v2:
  - The PE runs at full clock only when continuously fed (the projection
    phase hits 2.4GHz; the v2 attention loop sagged to ~1.4GHz because the
    per-group exp dependency starves it ~200ns per group and the clock
    drops). The output projection of chunk c-1 is therefore INTERLEAVED
    into chunk c's attention stream as always-ready filler work.
  - exp is split across engines: groups 0,2,4,6 use exact Act-engine Exp;
    groups 1,3,5,7 use a DVE fast-exp (int16(s*23.083+16249.64) bit
    pattern read as bf16 ~= exp(s/8), 1.7% mean err; washes out in the
    2048-way softmax averages - measured end-to-end 7e-3).
  - 1/sqrt(DK) folds into the exp scale (and the fast-exp multiplier).
  - Softmax denominator comes free from 64 ones-columns appended to V
    (PSUM rows 64..127 of the ctx accumulator); its reciprocal is
    exp(-ln(den)) on Act - ln/exp share one activation table (no reload),
    vs 3.4us per tile for DVE RECIPROCAL.
  - x arrives host-pre-transposed in bf16; V is computed directly in
    [t,(h,v)] layout (stationary = xT tile) - no PE transposes at all.
  - Q/K-projection PSUM evacuation alternates Act (Identity+bias) / DVE
    (tensor_scalar_add); outproj evacuation alternates Act/DVE copies;
    bo is added by the host during the gather.
"""
import sys

if "/opt/trn_rl_repo" not in sys.path:
    sys.path.insert(0, "/opt/trn_rl_repo")

import ml_dtypes
import numpy as np

import bass_rust
import concourse.bass as bass
import concourse.mybir as mybir
import concourse.tile as tile
from concourse.bass_utils import run_bass_kernel_spmd
from concourse.vector_clock import ScopedClock

F32 = mybir.dt.float32
BF16 = mybir.dt.bfloat16
I16 = mybir.dt.int16
AF = mybir.ActivationFunctionType
Alu = mybir.AluOpType
BF16NP = ml_dtypes.bfloat16

B, S, D = 2, 2048, 1024
H, DK, DV = 16, 64, 64
HL = 4          # heads per core
NPAIR = 2
ST = S // 128   # 16
DT = D // 128   # 8
SC = 512        # attention s-chunk
NSC = S // SC   # 4
N_CORES = 8

# int16(score*FEXP_A + FEXP_B) bit pattern read as bf16 ~= exp(score/8)
FEXP_A = 0.125 * 128.0 * 1.4426950408889634
FEXP_B = 16249.64
ACT_GROUPS = (0, 2, 4, 6)   # exact Act exp; remaining groups use DVE fastexp


class _TileContextSplitDrain(tile.TileContext):
    """Walrus in this container rejects ANY instruction carrying >1 sem wait
    ("Too many sync wait commands"). Post-lowering, sweep every basic block
    and move surplus waits onto injected EventSemaphore carrier instructions
    placed immediately before the over-subscribed instruction (same engine,
    same program point - semantics unchanged)."""

    _MAXW = 1

    def _split_excess_waits(self):
        nc = self.nc
        for fn in nc.m.functions:
            for bb in fn.blocks:
                insts = bb.instructions
                new_list = []
                changed = False
                for ins in insts:
                    si = ins.sync_info
                    waits = list(si.on_wait) if si is not None and si.on_wait else []
                    if len(waits) > self._MAXW:
                        changed = True
                        extra, keep = waits[:-self._MAXW], waits[-self._MAXW:]
                        for k in range(0, len(extra), self._MAXW):
                            chunk = extra[k:k + self._MAXW]
                            ev = mybir.InstEventSemaphore(
                                name=f"wsplit_{nc.next_id()}", ins=[], outs=[]
                            )
                            ev.engine = ins.engine
                            ev.sync_info = bass_rust.SyncInfo(
                                on_wait=chunk, on_update=[]
                            )
                            nc.register_instruction(ev, overwrite=True)
                            new_list.append(ev)
                        ins.sync_info = bass_rust.SyncInfo(
                            on_wait=keep,
                            on_update=list(si.on_update) if si.on_update else [],
                        )
                    new_list.append(ins)
                if changed:
                    insts[:] = new_list

    def _drain_and_barrier(self, tick_clock, wait_clock):
        self._split_excess_waits()
        ticks = list(tick_clock.global_clock)
        for p, t in enumerate(ticks):
            if t <= 0:
                continue
            v = bass_rust.VectorClock()
            v.require_at_least(p, t)
            d = self.nc.sync.drain()
            wait_clock.add_sem_waits(d.ins, ScopedClock({None: v}))
        self.nc.all_engine_barrier()
        popped = self.nc._tile_sem_poison_stack.pop()
        assert popped is self._sem_poison
        self.nc.clear_and_free_semaphores(list(self.sems.allocated().values()))
        self.nc.all_engine_barrier()


def build_nc() -> bass.Bass:
    nc = bass.Bass()

    # all inputs partition-major: row p holds everything partition p needs,
    # contiguously, so each dma_start is 128 large descriptors
    xT_d = nc.dram_tensor("xT", [128, NSC * DT * 512], BF16, kind="ExternalInput")
    wqk_d = nc.dram_tensor("wqk", [128, 4 * DT * 128], BF16, kind="ExternalInput")
    wv_d = nc.dram_tensor("wv", [128, DT * HL * DV], BF16, kind="ExternalInput")
    wo_d = nc.dram_tensor("wo", [128, NPAIR * D], BF16, kind="ExternalInput")
    bqk_d = nc.dram_tensor("bqk", [128, 4], F32, kind="ExternalInput")
    bv_d = nc.dram_tensor("bv", [1, HL * DV], F32, kind="ExternalInput")
    # partials ship as bf16: host gather sums in fp32 and adds bo; halves
    # the 8MB output DMA (quantization adds ~0.1-0.3% to a 2e-2 budget)
    out_d = nc.dram_tensor("out", [S, D], BF16, kind="ExternalOutput")

    with _TileContextSplitDrain(nc) as tc:
        with (
            tc.tile_pool(name="const", bufs=1) as constp,
            tc.tile_pool(name="persist", bufs=1) as pers,
        ):
            wqk_sb = constp.tile([128, DT, 4 * 128], BF16, tag="wqk")
            wv_sb = constp.tile([128, DT, HL * DV], BF16, tag="wv")
            wo_sb = constp.tile([128, NPAIR, D], BF16, tag="wo")
            bqk_sb = constp.tile([128, 4], F32, tag="bqk")
            bv_rep = constp.tile([128, HL * DV], F32, tag="bv_rep")
            xT_sb = pers.tile([128, DT, S], BF16, tag="xT")
            QT = pers.tile([128, NPAIR, S], BF16, tag="QT")
            KT = pers.tile([128, NPAIR, S], BF16, tag="KT")
            ctxT = pers.tile([128, NPAIR, S], BF16, tag="ctxT")
            V_aug = pers.tile([128, ST, HL, 128], BF16, tag="V_aug")

            def dma_xT(sc):
                nc.sync.dma_start(
                    out=xT_sb[:, :, sc * 512:(sc + 1) * 512],
                    in_=xT_d.rearrange("p (c d s) -> p c d s", c=NSC, d=DT)[:, sc, :, :],
                )

            wqk_r = wqk_d.rearrange("p (cb d c) -> p cb d c", cb=4, d=DT)
            dma_xT(0)
            nc.sync.dma_start(out=wqk_sb[:, :, 0:128], in_=wqk_r[:, 0, :, :])
            nc.sync.dma_start(out=wqk_sb[:, :, 128:256], in_=wqk_r[:, 1, :, :])
            dma_xT(1)
            nc.sync.dma_start(out=wqk_sb[:, :, 256:384], in_=wqk_r[:, 2, :, :])
            nc.sync.dma_start(out=wqk_sb[:, :, 384:512], in_=wqk_r[:, 3, :, :])
            nc.sync.dma_start(out=wv_sb, in_=wv_d[:, :])
            nc.sync.dma_start(out=bqk_sb, in_=bqk_d[:, :])
            nc.sync.dma_start(out=bv_rep, in_=bv_d[0:1, :].to_broadcast((128, HL * DV)))
            nc.vector.memset(V_aug[:, :, :, 64:128], 1.0)

            # ---------------- Phase 1: QKV projections ----------------------
            with (
                tc.tile_pool(name="qkps", bufs=5, space="PSUM") as qkp,
                tc.tile_pool(name="vps", bufs=3, space="PSUM") as vp,
            ):
                for sc in range(4):
                    if sc + 2 < 4:
                        dma_xT(sc + 2)
                    if sc == 1:
                        nc.sync.dma_start(out=wo_sb, in_=wo_d[:, :])
                    for proj, dest in ((0, QT), (1, KT)):
                        for pair in range(NPAIR):
                            col = (2 * proj + pair) * 128
                            ps = qkp.tile([128, 512], F32, tag="qkps")
                            for d in range(DT):
                                nc.tensor.matmul(
                                    ps[:, :],
                                    wqk_sb[:, d, col:col + 128],
                                    xT_sb[:, d, sc * 512:(sc + 1) * 512],
                                    start=(d == 0),
                                    stop=(d == DT - 1),
                                )
                            dst = dest[:, pair, sc * 512:(sc + 1) * 512]
                            bias = bqk_sb[:, 2 * proj + pair:2 * proj + pair + 1]
                            if pair == 0:
                                nc.scalar.activation(
                                    out=dst, in_=ps[:, :], func=AF.Identity,
                                    bias=bias, scale=1.0,
                                )
                            else:
                                nc.vector.tensor_scalar_add(dst, ps[:, :], bias)
                    for tt in range(4):
                        j = 4 * sc + tt
                        vps = vp.tile([128, HL * DV], F32, tag="vps")
                        for d in range(DT):
                            nc.tensor.matmul(
                                vps[:, :],
                                xT_sb[:, d, j * 128:(j + 1) * 128],
                                wv_sb[:, d, :],
                                start=(d == 0),
                                stop=(d == DT - 1),
                            )
                        nc.vector.tensor_add(
                            V_aug[:, j, :, 0:64],
                            vps.rearrange("p (h v) -> p h v", h=HL),
                            bv_rep.rearrange("p (h v) -> p h v", h=HL),
                        )

            # ---------------- Phase 2: attention with interleaved outproj ---
            with (
                tc.tile_pool(name="sps", bufs=2, space="PSUM") as spp,
                tc.tile_pool(name="cpp", bufs=2, space="PSUM") as cpp,
                tc.tile_pool(name="opp", bufs=2, space="PSUM") as opp,
                tc.tile_pool(name="ptp", bufs=8) as ptp,
                tc.tile_pool(name="recl", bufs=4) as recl,
                tc.tile_pool(name="recp", bufs=4) as recp,
                tc.tile_pool(name="otp", bufs=6) as otp,
            ):
                pending = []   # outproj steps of the previous chunk
                ot_ref = [None]

                def outproj_step(st, dc):
                    if dc == 0:
                        ot_ref[0] = otp.tile([128, D], BF16, name="ot", tag="ot")
                    ot = ot_ref[0]
                    ops_t = opp.tile([128, 512], F32, name="ops", tag="ops")
                    for pair in range(NPAIR):
                        nc.tensor.matmul(
                            ops_t[:, :],
                            ctxT[:, pair, st * 128:(st + 1) * 128],
                            wo_sb[:, pair, dc * 512:(dc + 1) * 512],
                            start=(pair == 0),
                            stop=(pair == NPAIR - 1),
                        )
                    if dc == 0:
                        nc.scalar.copy(ot[:, 0:512], ops_t[:, :])
                    else:
                        nc.vector.tensor_copy(ot[:, 512:1024], ops_t[:, :])
                        nc.sync.dma_start(
                            out=out_d[st * 128:(st + 1) * 128, :], in_=ot
                        )

                for c in range(NSC):
                    for pair in range(NPAIR):
                        for e in range(2):
                            h = 2 * pair + e
                            base = 64 * e
                            cp = cpp.tile([128, SC], F32, name="cp", tag="cp")
                            sps = {}

                            def emit_scores(g, _pair=pair, _base=base, _c=c):
                                sp = spp.tile([128, 2, SC], F32, name="sp", tag="sp")
                                for k in range(2):
                                    j = 2 * g + k
                                    nc.tensor.matmul(
                                        sp[:, k, :],
                                        KT[_base:_base + 64, _pair, j * 128:(j + 1) * 128],
                                        QT[_base:_base + 64, _pair, _c * SC:(_c + 1) * SC],
                                        start=True,
                                        stop=True,
                                    )
                                sps[g] = sp

                            emit_scores(0)
                            emit_scores(1)
                            for g in range(8):
                                if g + 2 < 8:
                                    emit_scores(g + 2)
                                sp = sps.pop(g)
                                pt = ptp.tile([128, 2, SC], BF16, tag="pt")
                                if g in ACT_GROUPS:
                                    nc.scalar.activation(
                                        out=pt[:, :, :], in_=sp[:, :, :],
                                        func=AF.Exp, scale=0.125,
                                    )
                                else:
                                    nc.vector.tensor_scalar(
                                        pt.bitcast(I16), sp[:, :, :],
                                        FEXP_A, FEXP_B, Alu.mult, Alu.add,
                                    )
                                for k in range(2):
                                    j = 2 * g + k
                                    nc.tensor.matmul(
                                        cp[:, :],
                                        V_aug[:, j, h, :],
                                        pt[:, k, :],
                                        start=(g == 0 and k == 0),
                                        stop=(g == 7 and k == 1),
                                    )
                                if g % 2 == 1 and pending:
                                    pending.pop(0)()
                            # 1/den = exp(-ln(den)); den sits on psum rows
                            # 64..127 via the ones-columns of V_aug
                            rl = recl.tile([64, SC], F32, tag="rl")
                            nc.scalar.activation(out=rl, in_=cp[64:128, :], func=AF.Ln)
                            rec = recp.tile([64, SC], F32, tag="rec")
                            nc.scalar.activation(out=rec, in_=rl, func=AF.Exp, scale=-1.0)
                            nc.vector.tensor_mul(
                                ctxT[base:base + 64, pair, c * SC:(c + 1) * SC],
                                cp[0:64, :],
                                rec,
                            )
                    for st in range(4 * c, 4 * c + 4):
                        for dc in range(2):
                            pending.append(
                                lambda _st=st, _dc=dc: outproj_step(_st, _dc)
                            )
                while pending:
                    pending.pop(0)()

    return nc


_NC_CACHE = None


def get_nc() -> bass.Bass:
    global _NC_CACHE
    if _NC_CACHE is None:
        _NC_CACHE = build_nc()
    return _NC_CACHE


def prep_in_maps(hidden_state, Wq, bq, Wk, bk, Wv, bv, Wo, bo):
    hidden_state = np.asarray(hidden_state, np.float32)
    Wq, bq = np.asarray(Wq, np.float32), np.asarray(bq, np.float32)
    Wk, bk = np.asarray(Wk, np.float32), np.asarray(bk, np.float32)
    Wv, bv = np.asarray(Wv, np.float32), np.asarray(bv, np.float32)
    Wo, bo = np.asarray(Wo, np.float32), np.asarray(bo, np.float32)

    in_maps = []
    for core in range(N_CORES):
        b, g = core // 4, core % 4
        hs = slice(HL * g, HL * (g + 1))
        # [D, C] -> [128, DT, C]: row p holds d-tiles d*128+p contiguously
        def pmaj(a):
            return np.ascontiguousarray(
                a.reshape(DT, 128, -1).transpose(1, 0, 2)
            ).reshape(128, -1)

        xT = hidden_state[b].T.astype(BF16NP)
        # [128, NSC, DT, 512]
        xT = np.ascontiguousarray(
            xT.reshape(DT, 128, NSC, 512).transpose(1, 2, 0, 3)
        ).reshape(128, -1)
        cols = []
        for wmat in (Wq[hs], Wk[hs]):
            for pair in range(NPAIR):
                cols.append(
                    wmat[2 * pair:2 * pair + 2].transpose(1, 0, 2).reshape(D, 128)
                )
        # [D, 4*128] -> [128, colblk, DT, 128] partition-major per block
        wqk = np.ascontiguousarray(
            np.concatenate(cols, axis=1).astype(BF16NP)
            .reshape(DT, 128, 4, 128).transpose(1, 2, 0, 3)
        ).reshape(128, -1)
        wv_g = pmaj(Wv[hs].transpose(1, 0, 2).reshape(D, HL * DV).astype(BF16NP))
        wo_g = np.ascontiguousarray(
            Wo[HL * DV * g: HL * DV * (g + 1)].astype(BF16NP)
            .reshape(NPAIR, 128, D).transpose(1, 0, 2)
        ).reshape(128, -1)
        bqk_cols = []
        for bvec in (bq[hs], bk[hs]):
            for pair in range(NPAIR):
                bqk_cols.append(bvec[2 * pair:2 * pair + 2].reshape(128))
        bqk = np.stack(bqk_cols, axis=1).astype(np.float32)
        in_maps.append({
            "xT": xT,
            "wqk": np.ascontiguousarray(wqk),
            "wv": np.ascontiguousarray(wv_g),
            "wo": wo_g,
            "bqk": np.ascontiguousarray(bqk),
            "bv": np.ascontiguousarray(bv[hs].reshape(1, HL * DV)),
        })
    return in_maps


_BO = None


def gather(results):
    """Sum the 4 row-parallel partials per batch, then add bo."""
    out = np.empty((B, S, D), np.float32)
    for b in range(B):
        acc = results[4 * b]["out"].astype(np.float32)
        for g in range(1, 4):
            acc = acc + results[4 * b + g]["out"]
        out[b] = acc + _BO[None, :]
    return out


def kernel(**inputs) -> np.ndarray:
    global _BO
    _BO = np.asarray(inputs["bo"], np.float32)
    nc = get_nc()
    in_maps = prep_in_maps(**inputs)
    res = run_bass_kernel_spmd(nc, in_maps, core_ids=list(range(N_CORES)))
    return gather(res.results)
